# revision 5
# baseline (speedup 1.0000x reference)
"""DeciLM-MoE (routed top-2-of-8 experts + shared expert) on 8 trn2 NeuronCores.

Sharding: expert-parallel for the routed experts (core c owns expert c) and
tensor-parallel over the shared-expert intermediate dim (core c owns rows
c*512:(c+1)*512).  Router is computed (replicated) on every core on-device.
Each core produces a partial output (its expert's routed contribution + its
shared-expert slice); the host sums the 8 partials (the expert-parallel
all-reduce) and transposes back.

On-device layout: activations ride with the FEATURE dim on partitions and
tokens on the free dim ("transposed" orientation) so the gate->silu->down
chain needs no on-chip transposes:
    G^T[i,t]   = sum_h gw^T[h,i] * x^T[h,t]      (stationary gw, moving x^T)
    out^T[h,t] = sum_i dw^T[i,h] * hmid^T[i,t]
Top-2 masking is done with max/2nd-max compares; the per-token scale row is
broadcast to 128 partitions with a one-hot select matmul.  All big matmuls
run as float32r (full-rate fp32 streaming mode, free dim 512).
"""

import sys

sys.path.insert(0, "/opt/trn_rl_repo")

import numpy as np

import concourse.bass as bass
import concourse.bass_isa as bass_isa
import concourse.mybir as mybir
import concourse.tile as tile
from concourse import bacc, bass_utils
from concourse.mybir import dt

ALU = mybir.AluOpType
ACTF = mybir.ActivationFunctionType

# Problem geometry (fixed by the reference).
B, S, H, E, I, IS = 2, 2048, 2048, 8, 1024, 4096
T = B * S                     # 4096 tokens
P = 128
N_CORES = 8
SSH = IS // N_CORES           # shared-expert slice per core: 512

TC = 512                      # token chunk = psum free dim
SUP = 2                       # chunks per super-chunk (weight reuse factor)
NSUP = T // (TC * SUP)        # 4 super-chunks
HT = H // P                   # 16 h-tiles (contraction for gate/up/router)
IT = I // P                   # 8 i-tiles
STT = SSH // P                # 4 shared i-tiles

F32R = dt.float32r
BIG = 1.0e30

USE_GPSIMD_BCAST = True


def _r(ap):
    return ap.bitcast(F32R)


def build_device_program(nc):
    """Emit the SPMD program (identical on all cores; per-core data differs)."""
    f32 = dt.float32
    xt = nc.dram_tensor("xt", [NSUP, P, HT * SUP * TC], F32R, kind="ExternalInput").ap()
    gwuw = nc.dram_tensor("gwuw", [2 * IT, P, HT * P], F32R, kind="ExternalInput").ap()
    dw = nc.dram_tensor("dw", [HT, P, IT * P], F32R, kind="ExternalInput").ap()
    sgsu = nc.dram_tensor("sgsu", [2 * STT, P, HT * P], F32R, kind="ExternalInput").ap()
    sdw = nc.dram_tensor("sdw", [HT, P, STT * P], F32R, kind="ExternalInput").ap()
    rw = nc.dram_tensor("rw", [P, HT * E], f32, kind="ExternalInput").ap()
    sel = nc.dram_tensor("sel", [E, P], f32, kind="ExternalInput").ap()
    ones = nc.dram_tensor("ones", [1, P], f32, kind="ExternalInput").ap()
    outT = nc.dram_tensor("outT", [H, T], f32, kind="ExternalOutput").ap()
    logT = nc.dram_tensor("logT", [E, T], f32, kind="ExternalOutput").ap()

    with tile.TileContext(nc) as tc:
        with (
            tc.tile_pool(name="const", bufs=1) as const_pool,
            tc.tile_pool(name="xt", bufs=1) as xt_pool,
            tc.tile_pool(name="w", bufs=3) as w_pool,
            tc.tile_pool(name="dwp", bufs=2) as dw_pool,
            tc.tile_pool(name="sdwp", bufs=2) as sdw_pool,
            tc.tile_pool(name="hmid", bufs=1) as hmid_pool,
            tc.tile_pool(name="tmp", bufs=2) as tmp_pool,
            tc.tile_pool(name="spool", bufs=2) as s_pool,
            tc.tile_pool(name="small", bufs=1) as small_pool,
            tc.tile_pool(name="outp", bufs=3) as out_pool,
            tc.tile_pool(name="pgu", bufs=2, space="PSUM") as pgu_pool,
            tc.tile_pool(name="pdown", bufs=2, space="PSUM") as pdown_pool,
            tc.tile_pool(name="pmisc", bufs=2, space="PSUM") as pmisc_pool,
        ):
            rw_sb = const_pool.tile([P, HT * E], f32, tag="rw")
            nc.sync.dma_start(rw_sb[:], rw[:])
            sel_sb = const_pool.tile([E, P], f32, tag="sel")
            nc.sync.dma_start(sel_sb[:], sel[:])
            ones_sb = const_pool.tile([1, P], f32, tag="ones")
            nc.sync.dma_start(ones_sb[:], ones[:])

            for sp in range(NSUP):
                xt_sb = xt_pool.tile([P, HT * SUP * TC], F32R, tag="xt")
                nc.sync.dma_start(xt_sb[:], xt[sp])

                def xtsl(hh, ch):
                    off = hh * (SUP * TC) + ch * TC
                    return xt_sb[:, off : off + TC]

                # ---------- routing scales, one per chunk ----------
                s_tiles = []
                for ch in range(SUP):
                    g = sp * SUP + ch
                    pm_r = pmisc_pool.tile([E, TC], f32, tag="pm")
                    for hh in range(HT):
                        nc.tensor.matmul(
                            pm_r[:],
                            rw_sb[:, hh * E : (hh + 1) * E],
                            xtsl(hh, ch).bitcast(f32),
                            start=(hh == 0),
                            stop=(hh == HT - 1),
                        )
                    logits = small_pool.tile([E, TC], f32, tag="logits")
                    nc.scalar.copy(logits[:], pm_r[:])
                    nc.sync.dma_start(logT[:, g * TC : (g + 1) * TC], logits[:])

                    m1b = small_pool.tile([E, TC], f32, tag="m1b")
                    nc.gpsimd.partition_all_reduce(
                        m1b[:], logits[:], channels=E, reduce_op=bass_isa.ReduceOp.max
                    )

                    eq = small_pool.tile([E, TC], f32, tag="eq")
                    nc.vector.tensor_tensor(eq[:], logits[:], m1b[:], op=ALU.is_ge)
                    masked = small_pool.tile([E, TC], f32, tag="mk")
                    nc.vector.scalar_tensor_tensor(
                        masked[:], eq[:], -BIG, logits[:], op0=ALU.mult, op1=ALU.add
                    )

                    m2b = small_pool.tile([E, TC], f32, tag="m2b")
                    nc.gpsimd.partition_all_reduce(
                        m2b[:], masked[:], channels=E, reduce_op=bass_isa.ReduceOp.max
                    )

                    mask2 = small_pool.tile([E, TC], f32, tag="mask2")
                    nc.vector.tensor_tensor(mask2[:], logits[:], m2b[:], op=ALU.is_ge)
                    sig = small_pool.tile([E, TC], f32, tag="sig")
                    nc.scalar.activation(sig[:], logits[:], ACTF.Sigmoid)
                    sc8 = small_pool.tile([E, TC], f32, tag="sc8")
                    nc.vector.tensor_tensor(sc8[:], sig[:], mask2[:], op=ALU.mult)

                    pm_s = pmisc_pool.tile([P, TC], f32, tag="pm")
                    nc.tensor.matmul(pm_s[:], sel_sb[:], sc8[:], start=True, stop=True)
                    s_sb = s_pool.tile([P, TC], f32, tag=f"s{ch}")
                    nc.scalar.copy(s_sb[:], pm_s[:])
                    s_tiles.append(s_sb)

                # ---------- routed gate/up -> hmid_r ----------
                hmr = [[None] * IT for _ in range(SUP)]
                hms = [[None] * STT for _ in range(SUP)]
                for i in range(IT):
                    wg = w_pool.tile([P, HT * P], F32R, tag="w")
                    nc.sync.dma_start(wg[:], gwuw[2 * i])
                    wu = w_pool.tile([P, HT * P], F32R, tag="w")
                    nc.sync.dma_start(wu[:], gwuw[2 * i + 1])
                    for ch in range(SUP):
                        pg = pgu_pool.tile([P, TC], f32, tag="pg")
                        for hh in range(HT):
                            nc.tensor.matmul(
                                pg[:],
                                wg[:, hh * P : (hh + 1) * P],
                                xtsl(hh, ch),
                                start=(hh == 0),
                                stop=(hh == HT - 1),
                            )
                        pu = pgu_pool.tile([P, TC], f32, tag="pu")
                        for hh in range(HT):
                            nc.tensor.matmul(
                                pu[:],
                                wu[:, hh * P : (hh + 1) * P],
                                xtsl(hh, ch),
                                start=(hh == 0),
                                stop=(hh == HT - 1),
                            )
                        tg = tmp_pool.tile([P, TC], f32, tag="tg")
                        nc.vector.tensor_tensor(tg[:], pg[:], s_tiles[ch][:], op=ALU.mult)
                        tgs = tmp_pool.tile([P, TC], f32, tag="tgs")
                        nc.scalar.activation(tgs[:], tg[:], ACTF.Silu)
                        tu = tmp_pool.tile([P, TC], f32, tag="tu")
                        nc.vector.tensor_tensor(tu[:], pu[:], s_tiles[ch][:], op=ALU.mult)
                        hm = hmid_pool.tile([P, TC], F32R, tag=f"hr{ch}_{i}")
                        nc.vector.tensor_tensor(hm[:], tgs[:], tu[:], op=ALU.mult)
                        hmr[ch][i] = hm

                # ---------- shared gate/up -> hmid_s ----------
                for st_ in range(STT):
                    wsg = w_pool.tile([P, HT * P], F32R, tag="w")
                    nc.sync.dma_start(wsg[:], sgsu[2 * st_])
                    wsu = w_pool.tile([P, HT * P], F32R, tag="w")
                    nc.sync.dma_start(wsu[:], sgsu[2 * st_ + 1])
                    for ch in range(SUP):
                        psg = pgu_pool.tile([P, TC], f32, tag="pg")
                        for hh in range(HT):
                            nc.tensor.matmul(
                                psg[:],
                                wsg[:, hh * P : (hh + 1) * P],
                                xtsl(hh, ch),
                                start=(hh == 0),
                                stop=(hh == HT - 1),
                            )
                        psu = pgu_pool.tile([P, TC], f32, tag="pu")
                        for hh in range(HT):
                            nc.tensor.matmul(
                                psu[:],
                                wsu[:, hh * P : (hh + 1) * P],
                                xtsl(hh, ch),
                                start=(hh == 0),
                                stop=(hh == HT - 1),
                            )
                        tss = tmp_pool.tile([P, TC], f32, tag="tgs")
                        nc.scalar.activation(tss[:], psg[:], ACTF.Silu)
                        hs = hmid_pool.tile([P, TC], F32R, tag=f"hs{ch}_{st_}")
                        nc.vector.tensor_tensor(hs[:], tss[:], psu[:], op=ALU.mult)
                        hms[ch][st_] = hs

                # ---------- down (routed + shared accumulate in psum) ----------
                for ht_ in range(HT):
                    wd = dw_pool.tile([P, IT * P], F32R, tag="wd")
                    nc.sync.dma_start(wd[:], dw[ht_])
                    wsd = sdw_pool.tile([P, STT * P], F32R, tag="wsd")
                    nc.sync.dma_start(wsd[:], sdw[ht_])
                    for ch in range(SUP):
                        g = sp * SUP + ch
                        po = pdown_pool.tile([P, TC], f32, tag="po")
                        for ik in range(IT):
                            nc.tensor.matmul(
                                po[:],
                                wd[:, ik * P : (ik + 1) * P],
                                hmr[ch][ik][:],
                                start=(ik == 0),
                                stop=False,
                            )
                        for sk in range(STT):
                            nc.tensor.matmul(
                                po[:],
                                wsd[:, sk * P : (sk + 1) * P],
                                hms[ch][sk][:],
                                start=False,
                                stop=(sk == STT - 1),
                            )
                        ot = out_pool.tile([P, TC], f32, tag="ot")
                        nc.scalar.copy(ot[:], po[:])
                        nc.sync.dma_start(
                            outT[ht_ * P : (ht_ + 1) * P, g * TC : (g + 1) * TC], ot[:]
                        )
    nc.compile()
    return nc


def _col_tiles(wT):
    """[K, M] (contraction-major) -> [M/P, P, K/P * P] stationary col-tile layout.

    tile[m][p, kk*P + q] = wT[kk*P + p, m*P + q]"""
    Kd, Md = wT.shape
    kt, mt = Kd // P, Md // P
    return np.ascontiguousarray(
        wT.reshape(kt, P, mt, P).transpose(2, 1, 0, 3)
    ).reshape(mt, P, kt * P)


def _interleave(a, b):
    return np.ascontiguousarray(
        np.concatenate([a[:, None], b[:, None]], axis=1)
    ).reshape(2 * a.shape[0], a.shape[1], a.shape[2])


def prepare_inputs(hidden_states, router_w, gate_w, up_w, down_w,
                   shared_gate_w, shared_up_w, shared_down_w):
    f = lambda a: np.asarray(a, dtype=np.float32)
    x = f(hidden_states).reshape(T, H)
    xT = np.ascontiguousarray(x.T)
    xt_host = np.ascontiguousarray(
        xT.reshape(HT, P, NSUP, SUP * TC).transpose(2, 1, 0, 3)
    ).reshape(NSUP, P, HT * SUP * TC)

    rwT = np.ascontiguousarray(f(router_w).T)                       # [H, E]
    rw_host = np.ascontiguousarray(
        rwT.reshape(HT, P, E).transpose(1, 0, 2)
    ).reshape(P, HT * E)
    ones_host = np.ones((1, P), np.float32)

    gate_w, up_w, down_w = f(gate_w), f(up_w), f(down_w)
    sgw, suw, sdw_full = f(shared_gate_w), f(shared_up_w), f(shared_down_w)

    in_maps = []
    for c in range(N_CORES):
        gw_t = _col_tiles(np.ascontiguousarray(gate_w[c].T))        # [8,128,2048]
        uw_t = _col_tiles(np.ascontiguousarray(up_w[c].T))
        dw_t = _col_tiles(np.ascontiguousarray(down_w[c].T))        # [16,128,1024]
        sg_t = _col_tiles(np.ascontiguousarray(sgw[c * SSH : (c + 1) * SSH].T))
        su_t = _col_tiles(np.ascontiguousarray(suw[c * SSH : (c + 1) * SSH].T))
        sd_t = _col_tiles(np.ascontiguousarray(sdw_full[:, c * SSH : (c + 1) * SSH].T))
        sel_host = np.zeros((E, P), np.float32)
        sel_host[c, :] = 1.0
        in_maps.append({
            "xt": xt_host,
            "gwuw": _interleave(gw_t, uw_t),
            "dw": dw_t,
            "sgsu": _interleave(sg_t, su_t),
            "sdw": sd_t,
            "rw": rw_host,
            "sel": sel_host,
            "ones": ones_host,
        })
    return in_maps


def assemble_outputs(results):
    acc = np.zeros((H, T), np.float64)
    for c in range(N_CORES):
        acc += results[c]["outT"]
    out = acc.astype(np.float32).T.reshape(B, S, H)
    logits = np.ascontiguousarray(results[0]["logT"].T).reshape(B, S, E)
    return out, logits


_CACHED_NC = None


def kernel(hidden_states, router_w, gate_w, up_w, down_w,
           shared_gate_w, shared_up_w, shared_down_w):
    global _CACHED_NC
    in_maps = prepare_inputs(hidden_states, router_w, gate_w, up_w, down_w,
                             shared_gate_w, shared_up_w, shared_down_w)
    if _CACHED_NC is None:
        nc = bacc.Bacc("TRN2", target_bir_lowering=False, debug=False,
                       num_devices=N_CORES)
        build_device_program(nc)
        _CACHED_NC = nc
    res = bass_utils.run_bass_kernel_spmd(
        _CACHED_NC, in_maps, core_ids=list(range(N_CORES))
    )
    return assemble_outputs(res.results)


# revision 8
# speedup vs baseline: 235.7239x; 235.7239x over previous
"""DeciLM-MoE (routed top-2-of-8 experts + shared expert) on 8 trn2 NeuronCores.

Sharding: expert-parallel for the routed experts (core c owns expert c) and
tensor-parallel over the shared-expert intermediate dim (core c owns rows
c*512:(c+1)*512).  Router is computed (replicated) on every core on-device.
Each core produces a partial output (its expert's routed contribution + its
shared-expert slice); the host sums the 8 partials (the expert-parallel
all-reduce) and transposes back.

On-device layout: activations ride with the FEATURE dim on partitions and
tokens on the free dim ("transposed" orientation) so the gate->silu->down
chain needs no on-chip transposes:
    G^T[i,t]   = sum_h gw^T[h,i] * x^T[h,t]      (stationary gw, moving x^T)
    out^T[h,t] = sum_i dw^T[i,h] * hmid^T[i,t]
Top-2 masking: gpsimd partition_all_reduce(max) gives the per-token max
broadcast over the 8 expert rows; masking the argmax and reducing again gives
the 2nd max; scale = sigmoid(logits) * (logits >= max2).  The per-token scale
row is broadcast to 128 partitions with a one-hot select matmul (the `sel`
input differs per core - that is how one SPMD program serves 8 experts).

Datatypes: gate/up/down/shared matmuls run as float32r (full PE rate at free
dim 512; a 20-bit 8-mantissa-bit format, so ~bf16-class noise, measured
6.6e-3 L2 rel err vs the fp32 reference).  The router matmul runs as plain
float32 (walrus 2-pass, ~16-bit effective mantissa) because top-2 selection
is sensitive to logit noise.  Cost model (TimelineSim): ~1.12 ms per core;
the dense-f32r PE roofline for this sharding is ~0.98 ms.
"""

import sys

sys.path.insert(0, "/opt/trn_rl_repo")

import ml_dtypes
import numpy as np

import concourse.bass as bass
import concourse.bass_isa as bass_isa
import concourse.mybir as mybir
import concourse.tile as tile
from concourse import bacc, bass_utils
from concourse.mybir import dt

ALU = mybir.AluOpType
ACTF = mybir.ActivationFunctionType

# Problem geometry (fixed by the reference).
B, S, H, E, I, IS = 2, 2048, 2048, 8, 1024, 4096
T = B * S                     # 4096 tokens
P = 128
N_CORES = 8
SSH = IS // N_CORES           # shared-expert slice per core: 512

TC = 512                      # token chunk = psum free dim
SUP = 2                       # chunks per super-chunk (weight reuse factor)
NSUP = T // (TC * SUP)        # 4 super-chunks
HT = H // P                   # 16 h-tiles (contraction for gate/up/router)
IT = I // P                   # 8 i-tiles
STT = SSH // P                # 4 shared i-tiles

F32R = dt.float32r
BF16 = dt.bfloat16
BIG = 1.0e30

USE_GPSIMD_BCAST = True


def _rne9(v):
    """Round fp32 to 8 explicit mantissa bits (fp32r lattice), round-to-nearest-even."""
    u = np.asarray(v, np.float32).view(np.uint32)
    r = (u + 0x3FFF + ((u >> 15) & 1)) & np.uint32(0xFFFF8000)
    return r.view(np.float32)


def build_device_program(nc):
    """Emit the SPMD program (identical on all cores; per-core data differs)."""
    f32 = dt.float32
    xt = nc.dram_tensor("xt", [NSUP, P, HT * SUP * TC], F32R, kind="ExternalInput").ap()
    gwuw = nc.dram_tensor("gwuw", [2 * IT, P, HT * P], F32R, kind="ExternalInput").ap()
    dw = nc.dram_tensor("dw", [HT, P, IT * P], F32R, kind="ExternalInput").ap()
    sgsu = nc.dram_tensor("sgsu", [2 * STT, P, HT * P], F32R, kind="ExternalInput").ap()
    sdw = nc.dram_tensor("sdw", [HT, P, STT * P], F32R, kind="ExternalInput").ap()
    rw = nc.dram_tensor("rw", [P, HT * E], f32, kind="ExternalInput").ap()
    sel = nc.dram_tensor("sel", [E, P], f32, kind="ExternalInput").ap()
    ones = nc.dram_tensor("ones", [1, P], f32, kind="ExternalInput").ap()
    outT = nc.dram_tensor("outT", [H, T], f32, kind="ExternalOutput").ap()
    logT = nc.dram_tensor("logT", [E, T], f32, kind="ExternalOutput").ap()

    with tile.TileContext(nc) as tc:
        with (
            tc.tile_pool(name="const", bufs=1) as const_pool,
            tc.tile_pool(name="xt", bufs=1) as xt_pool,
            tc.tile_pool(name="w", bufs=3) as w_pool,
            tc.tile_pool(name="dwp", bufs=2) as dw_pool,
            tc.tile_pool(name="sdwp", bufs=2) as sdw_pool,
            tc.tile_pool(name="hmid", bufs=1) as hmid_pool,
            tc.tile_pool(name="tmp", bufs=2) as tmp_pool,
            tc.tile_pool(name="spool", bufs=2) as s_pool,
            tc.tile_pool(name="small", bufs=1) as small_pool,
            tc.tile_pool(name="outp", bufs=3) as out_pool,
            tc.tile_pool(name="pgu", bufs=2, space="PSUM") as pgu_pool,
            tc.tile_pool(name="pdown", bufs=2, space="PSUM") as pdown_pool,
            tc.tile_pool(name="pmisc", bufs=2, space="PSUM") as pmisc_pool,
        ):
            rw_sb = const_pool.tile([P, HT * E], f32, tag="rw")
            nc.sync.dma_start(rw_sb[:], rw[:])
            sel_sb = const_pool.tile([E, P], f32, tag="sel")
            nc.sync.dma_start(sel_sb[:], sel[:])
            ones_sb = const_pool.tile([1, P], f32, tag="ones")
            nc.sync.dma_start(ones_sb[:], ones[:])

            for sp in range(NSUP):
                xt_sb = xt_pool.tile([P, HT * SUP * TC], F32R, tag="xt")
                nc.sync.dma_start(xt_sb[:], xt[sp])

                def xtsl(hh, ch):
                    off = hh * (SUP * TC) + ch * TC
                    return xt_sb[:, off : off + TC]

                # ---------- routing scales, one per chunk ----------
                s_tiles = []
                for ch in range(SUP):
                    g = sp * SUP + ch
                    pm_r = pmisc_pool.tile([E, TC], f32, tag="pm")
                    for hh in range(HT):
                        nc.tensor.matmul(
                            pm_r[:],
                            rw_sb[:, hh * E : (hh + 1) * E],
                            xtsl(hh, ch).bitcast(f32),
                            start=(hh == 0),
                            stop=(hh == HT - 1),
                        )
                    logits = small_pool.tile([E, TC], f32, tag="logits")
                    nc.scalar.copy(logits[:], pm_r[:])
                    nc.sync.dma_start(logT[:, g * TC : (g + 1) * TC], logits[:])

                    m1b = small_pool.tile([E, TC], f32, tag="mb")
                    nc.gpsimd.partition_all_reduce(
                        m1b[:], logits[:], channels=E, reduce_op=bass_isa.ReduceOp.max
                    )

                    eq = small_pool.tile([E, TC], f32, tag="eq")
                    nc.vector.tensor_tensor(eq[:], logits[:], m1b[:], op=ALU.is_ge)
                    masked = small_pool.tile([E, TC], f32, tag="mk")
                    nc.vector.scalar_tensor_tensor(
                        masked[:], eq[:], -BIG, logits[:], op0=ALU.mult, op1=ALU.add
                    )

                    m2b = small_pool.tile([E, TC], f32, tag="mb")
                    nc.gpsimd.partition_all_reduce(
                        m2b[:], masked[:], channels=E, reduce_op=bass_isa.ReduceOp.max
                    )

                    mask2 = small_pool.tile([E, TC], f32, tag="mask2")
                    nc.vector.tensor_tensor(mask2[:], logits[:], m2b[:], op=ALU.is_ge)
                    sig = small_pool.tile([E, TC], f32, tag="sig")
                    nc.scalar.activation(sig[:], logits[:], ACTF.Sigmoid)
                    sc8 = small_pool.tile([E, TC], f32, tag="sc8")
                    nc.vector.tensor_tensor(sc8[:], sig[:], mask2[:], op=ALU.mult)

                    pm_s = pmisc_pool.tile([P, TC], f32, tag="pm")
                    nc.tensor.matmul(pm_s[:], sel_sb[:], sc8[:], start=True, stop=True)
                    s_sb = s_pool.tile([P, TC], f32, tag=f"s{ch}")
                    nc.scalar.copy(s_sb[:], pm_s[:])
                    s_tiles.append(s_sb)

                # ---------- routed gate/up -> hmid_r ----------
                hmr = [[None] * IT for _ in range(SUP)]
                hms = [[None] * STT for _ in range(SUP)]
                for i in range(IT):
                    wg = w_pool.tile([P, HT * P], F32R, tag="w")
                    nc.sync.dma_start(wg[:], gwuw[2 * i])
                    wu = w_pool.tile([P, HT * P], F32R, tag="w")
                    nc.sync.dma_start(wu[:], gwuw[2 * i + 1])
                    for ch in range(SUP):
                        pg = pgu_pool.tile([P, TC], f32, tag="pg")
                        for hh in range(HT):
                            nc.tensor.matmul(
                                pg[:],
                                wg[:, hh * P : (hh + 1) * P],
                                xtsl(hh, ch),
                                start=(hh == 0),
                                stop=(hh == HT - 1),
                            )
                        pu = pgu_pool.tile([P, TC], f32, tag="pu")
                        for hh in range(HT):
                            nc.tensor.matmul(
                                pu[:],
                                wu[:, hh * P : (hh + 1) * P],
                                xtsl(hh, ch),
                                start=(hh == 0),
                                stop=(hh == HT - 1),
                            )
                        tg = tmp_pool.tile([P, TC], f32, tag="tg")
                        nc.vector.tensor_tensor(tg[:], pg[:], s_tiles[ch][:], op=ALU.mult)
                        tgs = tmp_pool.tile([P, TC], f32, tag="tgs")
                        nc.scalar.activation(tgs[:], tg[:], ACTF.Silu)
                        tu = tmp_pool.tile([P, TC], f32, tag="tu")
                        nc.vector.tensor_tensor(tu[:], pu[:], s_tiles[ch][:], op=ALU.mult)
                        hm = hmid_pool.tile([P, TC], F32R, tag=f"hr{ch}_{i}")
                        nc.vector.tensor_tensor(hm[:], tgs[:], tu[:], op=ALU.mult)
                        hmr[ch][i] = hm

                # ---------- shared gate/up -> hmid_s ----------
                for st_ in range(STT):
                    wsg = w_pool.tile([P, HT * P], F32R, tag="w")
                    nc.sync.dma_start(wsg[:], sgsu[2 * st_])
                    wsu = w_pool.tile([P, HT * P], F32R, tag="w")
                    nc.sync.dma_start(wsu[:], sgsu[2 * st_ + 1])
                    for ch in range(SUP):
                        psg = pgu_pool.tile([P, TC], f32, tag="pg")
                        for hh in range(HT):
                            nc.tensor.matmul(
                                psg[:],
                                wsg[:, hh * P : (hh + 1) * P],
                                xtsl(hh, ch),
                                start=(hh == 0),
                                stop=(hh == HT - 1),
                            )
                        psu = pgu_pool.tile([P, TC], f32, tag="pu")
                        for hh in range(HT):
                            nc.tensor.matmul(
                                psu[:],
                                wsu[:, hh * P : (hh + 1) * P],
                                xtsl(hh, ch),
                                start=(hh == 0),
                                stop=(hh == HT - 1),
                            )
                        tss = tmp_pool.tile([P, TC], f32, tag="tgs")
                        nc.scalar.activation(tss[:], psg[:], ACTF.Silu)
                        hs = hmid_pool.tile([P, TC], F32R, tag=f"hs{ch}_{st_}")
                        nc.vector.tensor_tensor(hs[:], tss[:], psu[:], op=ALU.mult)
                        hms[ch][st_] = hs

                # ---------- down (routed + shared accumulate in psum) ----------
                for ht_ in range(HT):
                    wd = dw_pool.tile([P, IT * P], F32R, tag="wd")
                    nc.sync.dma_start(wd[:], dw[ht_])
                    wsd = sdw_pool.tile([P, STT * P], F32R, tag="wsd")
                    nc.sync.dma_start(wsd[:], sdw[ht_])
                    for ch in range(SUP):
                        g = sp * SUP + ch
                        po = pdown_pool.tile([P, TC], f32, tag="po")
                        for ik in range(IT):
                            nc.tensor.matmul(
                                po[:],
                                wd[:, ik * P : (ik + 1) * P],
                                hmr[ch][ik][:],
                                start=(ik == 0),
                                stop=False,
                            )
                        for sk in range(STT):
                            nc.tensor.matmul(
                                po[:],
                                wsd[:, sk * P : (sk + 1) * P],
                                hms[ch][sk][:],
                                start=False,
                                stop=(sk == STT - 1),
                            )
                        ot = out_pool.tile([P, TC], f32, tag="ot")
                        nc.scalar.copy(ot[:], po[:])
                        nc.sync.dma_start(
                            outT[ht_ * P : (ht_ + 1) * P, g * TC : (g + 1) * TC], ot[:]
                        )
    nc.compile()
    return nc


def _col_tiles(wT):
    """[K, M] (contraction-major) -> [M/P, P, K/P * P] stationary col-tile layout.

    tile[m][p, kk*P + q] = wT[kk*P + p, m*P + q]"""
    Kd, Md = wT.shape
    kt, mt = Kd // P, Md // P
    return np.ascontiguousarray(
        wT.reshape(kt, P, mt, P).transpose(2, 1, 0, 3)
    ).reshape(mt, P, kt * P)


def _interleave(a, b):
    return np.ascontiguousarray(
        np.concatenate([a[:, None], b[:, None]], axis=1)
    ).reshape(2 * a.shape[0], a.shape[1], a.shape[2])


def prepare_inputs(hidden_states, router_w, gate_w, up_w, down_w,
                   shared_gate_w, shared_up_w, shared_down_w):
    f = lambda a: np.asarray(a, dtype=np.float32)
    x = f(hidden_states).reshape(T, H)
    xT = np.ascontiguousarray(x.T)
    xt_host = np.ascontiguousarray(
        xT.reshape(HT, P, NSUP, SUP * TC).transpose(2, 1, 0, 3)
    ).reshape(NSUP, P, HT * SUP * TC)

    rwT = np.ascontiguousarray(f(router_w).T)                       # [H, E]
    rw_host = np.ascontiguousarray(
        rwT.reshape(HT, P, E).transpose(1, 0, 2)
    ).reshape(P, HT * E)
    ones_host = np.ones((1, P), np.float32)

    gate_w, up_w, down_w = f(gate_w), f(up_w), f(down_w)
    sgw, suw, sdw_full = f(shared_gate_w), f(shared_up_w), f(shared_down_w)

    in_maps = []
    for c in range(N_CORES):
        gw_t = _col_tiles(np.ascontiguousarray(gate_w[c].T))        # [8,128,2048]
        uw_t = _col_tiles(np.ascontiguousarray(up_w[c].T))
        dw_t = _col_tiles(np.ascontiguousarray(down_w[c].T))        # [16,128,1024]
        sg_t = _col_tiles(np.ascontiguousarray(sgw[c * SSH : (c + 1) * SSH].T))
        su_t = _col_tiles(np.ascontiguousarray(suw[c * SSH : (c + 1) * SSH].T))
        sd_t = _col_tiles(np.ascontiguousarray(sdw_full[:, c * SSH : (c + 1) * SSH].T))
        sel_host = np.zeros((E, P), np.float32)
        sel_host[c, :] = 1.0
        in_maps.append({
            "xt": xt_host,
            "gwuw": _interleave(gw_t, uw_t),
            "dw": dw_t,
            "sgsu": _interleave(sg_t, su_t),
            "sdw": sd_t,
            "rw": rw_host,
            "sel": sel_host,
            "ones": ones_host,
        })
    return in_maps


def assemble_outputs(results):
    acc = np.zeros((H, T), np.float64)
    for c in range(N_CORES):
        acc += results[c]["outT"]
    out = acc.astype(np.float32).T.reshape(B, S, H)
    logits = np.ascontiguousarray(results[0]["logT"].T).reshape(B, S, E)
    return out, logits


_CACHED_NC = None


def kernel(hidden_states, router_w, gate_w, up_w, down_w,
           shared_gate_w, shared_up_w, shared_down_w):
    global _CACHED_NC
    in_maps = prepare_inputs(hidden_states, router_w, gate_w, up_w, down_w,
                             shared_gate_w, shared_up_w, shared_down_w)
    if _CACHED_NC is None:
        nc = bacc.Bacc("TRN2", target_bir_lowering=False, debug=False,
                       num_devices=N_CORES)
        build_device_program(nc)
        _CACHED_NC = nc
    res = bass_utils.run_bass_kernel_spmd(
        _CACHED_NC, in_maps, core_ids=list(range(N_CORES))
    )
    return assemble_outputs(res.results)
